# revision 29
# baseline (speedup 1.0000x reference)
"""Trainium2 Bass kernel for nn_Attention_48000554500172.

16-head causal attention with RoPE (S=4096, D=2048, H=16, DH=128), sharded
over heads across 8 NeuronCores (2 heads/core, tensor parallel). Each core
computes its heads' QKV projections, RoPE, causal softmax attention and the
partial output projection; the 8 partial [S, D] outputs are summed on host
(the all-reduce of the sharding hint).

v3 design (v2 was 373us; TimelineSim ~341us):
- All matmul inputs bf16 (1 cycle/row on the PE at any moving size); PSUM
  accumulation fp32; output shipped bf16, partials summed f32 on host.
- RoPE entirely off the PE: rotate_half is a DVE stream_shuffle (pairwise
  partition swap within 32-blocks) and the sin table has its even-dh rows
  negated on host, so q' = q*cos + shuf(q)*sinA with two 4x-rate
  scalar_tensor_tensor muls + one add. (v2 used a PE permutation matmul per
  rope step + a PSUM-read mul.)
- V is produced directly in [keys, dh] layout: the v projection uses the x
  chunk as the STATIONARY operand (out partitions = s-tile) in 4 column
  quadrants, accumulating over d in PSUM; one strided copy lands v_sb.
  Kills the per-slice PE transposes and the extra drain.
- Softmax denominators: probs tiles fold on the DVE (bf16 quad trees via
  4x scalar_tensor_tensor + an f32 running master per head/group; diagonal
  blocks fold in-place at their causal widths). The 128-partition sum runs
  on the idle GPSIMD (partition_all_reduce, in-place), the reciprocal is a
  full-width DVE op, and ot = pv * (1/Z) reads it directly - no PE
  ones-matmuls, no PE broadcast matmul, no ACT bcs copies.
- Causal masking of diagonal blocks via precomputed 0/1 bf16 mask tiles
  applied with 4x-rate in-place scalar_tensor_tensor muls.
- Phase interleaving: the attention sweep is exp-paced on the ACT engine
  (612ns/block vs 426ns of PE sim+PV work), so the projection matmuls of
  slice g+1 and the tail+output projection of group g-1 are emitted as
  fill between the sweep blocks of group g, one matmul per sweep block
  (finer than v2's two, so fill lasts the whole last sweep). The sweep is
  software-pipelined 4 blocks deep (PV + Z-fold emitted 4 blocks after
  sim/exp).
- Startup: the first w and x chunks are split so the first matmul's
  operands arrive in ~2.5us; slice 0's first projection group is 6
  accumulators wide to cover the arriving stream.
- Endgame: the last group's output DMAs go out per 512-column block (the
  final block split in two) so the post-PE tail is one small copy + DMA.
"""
import math
import numpy as np
import ml_dtypes
from collections import deque
from contextlib import ExitStack

import concourse.bass as bass
import concourse.tile as tile
from concourse import bacc, mybir, bass_isa
from concourse.bass_utils import run_bass_kernel_spmd

D, H, DH = 2048, 16, 128
NCORES = 8
HPC = H // NCORES  # 2 heads per core
ROPE_BASE = 10000.0
SCALE = 1.0 / math.sqrt(DH)
F32 = mybir.dt.float32
BF16 = mybir.dt.bfloat16
Exp = mybir.ActivationFunctionType.Exp
Mult = mybir.AluOpType.mult
Add = mybir.AluOpType.add
BF = ml_dtypes.bfloat16

_BUILD_CACHE: dict = {}
TRACE = False
LAST_RESULT = None

# diagonal key-block p: query cols [128p, 512) are unmasked
DW = (512, 384, 256, 128)
DO = (0, 128, 256, 384)

# pairwise swap within 32-partition blocks (rotate_half partner)
SWAP_MASK = [m ^ 1 for m in range(32)]


def _build(S: int):
    assert S % 512 == 0
    NG = S // 512   # 8 query groups == s-slices
    ND = D // 128   # 16 contraction tiles
    NB = S // 128   # 32 key blocks

    nc = bacc.Bacc("TRN2", target_bir_lowering=False, debug=False)

    xT_d = nc.dram_tensor("xT", [D, S], BF16, kind="ExternalInput")
    w_d = nc.dram_tensor("wqkvT", [D, 768], BF16, kind="ExternalInput")
    wo_d = nc.dram_tensor("woT", [2 * DH, D], BF16, kind="ExternalInput")
    cs_d = nc.dram_tensor("cs", [128, 2 * S], BF16, kind="ExternalInput")
    cb_d = nc.dram_tensor("cb", [128, 2048], BF16, kind="ExternalInput")
    out_d = nc.dram_tensor("outp", [S, D], BF16, kind="ExternalOutput")
    import os as _os
    _dbg = _os.environ.get("DEBUG_DUMP") == "1"
    if _dbg:
        dbg_k0 = nc.dram_tensor("dbg_k0", [128, S], BF16, kind="ExternalOutput")
        dbg_k1 = nc.dram_tensor("dbg_k1", [128, S], BF16, kind="ExternalOutput")
        dbg_v = nc.dram_tensor("dbg_v", [128, (S // 128) * 256], BF16,
                               kind="ExternalOutput")

    with tile.TileContext(nc) as tc, ExitStack() as ctx:
        persist = ctx.enter_context(tc.tile_pool(name="persist", bufs=1))
        work = ctx.enter_context(tc.tile_pool(name="work", bufs=2))
        pacc = ctx.enter_context(tc.tile_pool(name="pacc", bufs=3, space="PSUM"))
        pshared = ctx.enter_context(tc.tile_pool(name="psh", bufs=3, space="PSUM"))
        ppv = ctx.enter_context(tc.tile_pool(name="ppv", bufs=2, space="PSUM"))

        # ---- persistent tiles ----
        w_r = persist.tile([128, ND * 768], BF16, tag="w", name="wr")
        wo_r = persist.tile([128, 2 * D], BF16, tag="wo", name="wor")
        kT = [persist.tile([128, S], BF16, tag=f"kT{h}", name=f"kT{h}")
              for h in range(2)]
        v_sb = persist.tile([128, NB * 256], BF16, tag="v", name="vsb")
        cb_t = persist.tile([128, 2048], BF16, tag="cb", name="cbt")
        masks = [cb_t[:, p * 512:p * 512 + DW[p]] for p in range(4)]

        # ---- DMA issue helpers ----
        def issue_x(g, interleave_w=False):
            """DMA x chunks (+cos/sin) for slice g; optionally interleave the
            16 w chunks + cb in d-consumption order (startup)."""
            xs = []
            for dd in range(ND // 2):
                t = work.tile([128, 1024], BF16, tag="xs", bufs=20, name="xs")
                src = xT_d.ap()[dd * 256:(dd + 1) * 256, g * 512:(g + 1) * 512]
                if interleave_w and dd == 0:
                    # split first pieces so the first matmul starts ~2.5us:
                    # w jt0-1 of d0..d127, then x d-step 0, then the rest
                    nc.sync.dma_start(w_r[:, 0:256], w_d.ap()[0:128, 0:256])
                    nc.sync.dma_start(
                        t[:, 0:256], src[0:128, 0:256])
                    nc.sync.dma_start(
                        t[:, 256:512], src[0:128, 256:512])
                    nc.sync.dma_start(w_r[:, 256:768], w_d.ap()[0:128, 256:768])
                    nc.sync.dma_start(
                        t[:, 512:1024], src[128:256, :])
                    nc.sync.dma_start(
                        w_r[:, 768:1536], w_d.ap()[128:256, :])
                    xs.append(t)
                    continue
                nc.sync.dma_start(
                    t[:].rearrange("b (a c) -> b a c", a=2),
                    src.rearrange("(a b) c -> b a c", a=2),
                )
                xs.append(t)
                if interleave_w:
                    # two w chunks in one strided DMA, consumption order
                    wdst = w_r[:, dd * 1536:(dd + 1) * 1536]
                    wsrc = w_d.ap()[dd * 256:(dd + 1) * 256, :]
                    nc.sync.dma_start(
                        wdst.rearrange("b (a c) -> b a c", a=2),
                        wsrc.rearrange("(a b) c -> b a c", a=2),
                    )
                    if dd == 3:
                        # masks: needed first at diag-block time (~13us)
                        nc.sync.dma_start(cb_t[:], cb_d.ap())
            cos = work.tile([128, 512], BF16, tag="cos", bufs=3, name="cos")
            sin = work.tile([128, 512], BF16, tag="sin", bufs=3, name="sin")
            nc.sync.dma_start(cos[:], cs_d.ap()[:, g * 512:(g + 1) * 512])
            nc.sync.dma_start(sin[:], cs_d.ap()[:, S + g * 512:S + (g + 1) * 512])
            return xs, cos, sin

        # ---- slice compute (projections + rope + v), as a fill generator ----
        # groups: ("q", jts), ("k", jts) are 512-wide accumulator passes;
        # ("v",) runs both heads' quad passes (stationary = x chunk).
        # Yields True when the step emitted PE work, False for DVE/ACT-only
        # steps; pull_fill only counts True yields so exp-paced sweep blocks
        # always receive real PE fill.
        PAIRS = (("q", (0, 1)), ("k", (2, 3)), ("v",))

        def slice_steps(g, xs, cos, sin, qts, pairs=PAIRS, startup=False):
            pend = []  # deferred post-steps, run with lag

            def rope_step(kind, hh, t_in):
                def run():
                    shf = work.tile([128, 512], BF16, tag="shf", bufs=3,
                                    name="shf")
                    nc.vector.stream_shuffle(shf[:], t_in[:], SWAP_MASK)
                    t1 = work.tile([128, 512], BF16, tag="t1", bufs=3,
                                   name="t1")
                    nc.vector.tensor_mul(t1[:], t_in[:], cos[:])
                    t2 = work.tile([128, 512], BF16, tag="t2", bufs=3,
                                   name="t2")
                    nc.vector.tensor_mul(t2[:], shf[:], sin[:])
                    if kind == "q":
                        nc.vector.tensor_add(qts[hh][:], t1[:], t2[:])
                    else:
                        nc.vector.tensor_add(
                            kT[hh][:, g * 512:(g + 1) * 512], t1[:], t2[:])
                return run

            def v_copy_step(hh, vq):
                def run():
                    dst = v_sb[:].rearrange("p (b x) -> p b x", x=256)[
                        :, 4 * g:4 * g + 4, hh * 128:(hh + 1) * 128]
                    src = vq[:].rearrange("p (b x) -> p b x", x=128)
                    nc.scalar.copy(dst, src)
                return run

            # startup: one 6-wide d-major pass using every idle PSUM bank
            # (q0,q1 pacc; k0,k1 psh+psh; vq0 psh... use pacc3+psh2+ppv1)
            if startup:
                POOL6 = ((pacc, "acc", 3), (pacc, "acc", 3),
                         (pacc, "acc", 3), (pshared, "sh", 3),
                         (pshared, "sh", 3), (ppv, "pv", 2))
                accs = [POOL6[i][0].tile([128, 512], F32, tag=POOL6[i][1],
                                         bufs=POOL6[i][2], name="acc6")
                        for i in range(6)]
                qk, vqs = accs[:4], accs[4:]
                for d in range(ND):
                    xr = xs[d // 2][:, (d % 2) * 512:(d % 2) * 512 + 512]
                    for jt in range(4):
                        if d == 0:
                            # column halves: the first half's x quarter
                            # arrives one DMA earlier (the second write hits
                            # pending-zero bytes, so this still zeroes)
                            nc.tensor.matmul(
                                qk[jt][:, 0:256],
                                w_r[:, jt * 128:(jt + 1) * 128],
                                xr[:, 0:256],
                                start=True, stop=False,
                                skip_group_check=True,
                            )
                            nc.tensor.matmul(
                                qk[jt][:, 256:512],
                                w_r[:, jt * 128:(jt + 1) * 128],
                                xr[:, 256:512],
                                start=False, stop=False,
                                skip_group_check=True,
                            )
                            yield True
                            continue
                        nc.tensor.matmul(
                            qk[jt][:],
                            w_r[:, d * 768 + jt * 128:d * 768 + (jt + 1) * 128],
                            xr,
                            start=(d == 0), stop=(d == ND - 1),
                            skip_group_check=True,
                        )
                        yield True
                    for hh in range(2):
                        for q in range(4):
                            # ONE start per psum bank row: start_tensor_calc
                            # arms pending-zero for the whole 2KB region, so
                            # later quads' first writes overwrite-then-clear
                            nc.tensor.matmul(
                                vqs[hh][:, q * 128:(q + 1) * 128],
                                xr[:, q * 128:(q + 1) * 128],
                                w_r[:, d * 768 + (4 + hh) * 128:
                                    d * 768 + (5 + hh) * 128],
                                start=(d == 0 and q == 0),
                                stop=(d == ND - 1 and q == 3),
                                skip_group_check=True,
                            )
                        yield True
                KINDS = ("q", "q", "k", "k")
                for jt in range(4):
                    hh = jt % 2
                    t_in = work.tile([128, 512], BF16, tag="tin", bufs=6,
                                     name="tin")
                    nc.scalar.copy(t_in[:, 0:256], qk[jt][:, 0:256])
                    nc.vector.tensor_copy(t_in[:, 256:512],
                                          qk[jt][:, 256:512])
                    pend.append(rope_step(KINDS[jt], hh, t_in))
                for hh in range(2):
                    pend.append(v_copy_step(hh, vqs[hh]))
                yield False
                while pend:
                    pend.pop(0)()
                    yield False
                return

            for grp in pairs:
                kind = grp[0]
                if kind == "v":
                    for hh in range(2):
                        vq = pacc.tile([128, 512], F32, tag="acc", bufs=3,
                                       name="vq")
                        for d in range(ND):
                            xr = xs[d // 2][:, (d % 2) * 512:
                                            (d % 2) * 512 + 512]
                            for q in range(4):
                                # ONE start per psum bank row (see startup)
                                nc.tensor.matmul(
                                    vq[:, q * 128:(q + 1) * 128],
                                    xr[:, q * 128:(q + 1) * 128],
                                    w_r[:, d * 768 + (4 + hh) * 128:
                                        d * 768 + (5 + hh) * 128],
                                    start=(d == 0 and q == 0),
                                    stop=(d == ND - 1 and q == 3),
                                    skip_group_check=True,
                                )
                            yield True
                        v_copy_step(hh, vq)()
                    continue
                jts = grp[1]
                acc = [pacc.tile([128, 512], F32, tag="acc", bufs=3,
                                 name="acc") for _ in jts]
                for d in range(ND):
                    xr = xs[d // 2][:, (d % 2) * 512:(d % 2) * 512 + 512]
                    for i, jt in enumerate(jts):
                        nc.tensor.matmul(
                            acc[i][:],
                            w_r[:, d * 768 + jt * 128:d * 768 + (jt + 1) * 128],
                            xr,
                            start=(d == 0), stop=(d == ND - 1),
                            skip_group_check=True,
                        )
                        yield True
                # drain the accs to SBUF (ACT + DVE in parallel); defer the
                # rope halves for later pulls
                for i, jt in enumerate(jts):
                    hh = jt % 2
                    t_in = work.tile([128, 512], BF16, tag="tin", bufs=6,
                                     name="tin")
                    nc.scalar.copy(t_in[:, 0:256], acc[i][:, 0:256])
                    nc.vector.tensor_copy(t_in[:, 256:512],
                                          acc[i][:, 256:512])
                    pend.append(rope_step(kind, hh, t_in))
                yield False
                while pend:
                    pend.pop(0)()
                    yield False
            while pend:
                pend.pop(0)()
                yield False

        # ---- group tail (Z allreduce/recip/ot) + output projection ----
        def finish_head(hh, master, half=None):
            """In-place: master <- 1/sum_partitions(master). half: 0/1 for
            column-half recip (last group's critical chain)."""
            if half is None:
                nc.gpsimd.partition_all_reduce(
                    master[:], master[:], channels=128,
                    reduce_op=bass_isa.ReduceOp.add)
                with nc.allow_low_precision(reason="recip of Z"):
                    nc.vector.reciprocal(master[:], master[:])
            else:
                sl = slice(half * 256, half * 256 + 256)
                with nc.allow_low_precision(reason="recip of Z"):
                    nc.vector.reciprocal(master[:, sl], master[:, sl])

        def tail_steps(g, pvz):
            last = g == NG - 1
            ots = []
            for hh in range(2):
                if hh == 0:
                    pv, rz = pvz[0]  # recip already done inline in sweep
                    ot = work.tile([128, 512], BF16, tag="ot0", bufs=3,
                                   name="ot0")
                    nc.vector.tensor_mul(ot[:], pv[:], rz[:])
                    ots.append(ot)
                    yield False
                    continue
                pv, master = pvz[1]
                if last:
                    # final critical chain: allreduce -> recip/ot per column
                    # half so the output projection starts on the first half
                    nc.gpsimd.partition_all_reduce(
                        master[:], master[:], channels=128,
                        reduce_op=bass_isa.ReduceOp.add)
                    ot = work.tile([128, 512], BF16, tag="ot1", bufs=3,
                                   name="ot1")
                    for half in range(2):
                        sl = slice(half * 256, half * 256 + 256)
                        finish_head(hh, master, half=half)
                        nc.vector.tensor_mul(ot[:, sl], pv[:, sl],
                                             master[:, sl])
                    ots.append(ot)
                    yield
                    continue
                finish_head(hh, master)
                yield False
                ot = work.tile([128, 512], BF16, tag="ot1", bufs=3,
                               name="ot1")
                nc.vector.tensor_mul(ot[:], pv[:], master[:])
                ots.append(ot)
                yield False
            pvz.append(ots)  # hand ots to op_steps via shared list

        def last_tail_op(g, pvz):
            pv0, rz0 = pvz[0]   # h0: 1/Z computed inline in the sweep
            pv1, m1 = pvz[1]
            ot0 = work.tile([128, 512], BF16, tag="ot0", bufs=3, name="ot0")
            nc.vector.tensor_mul(ot0[:], pv0[:], rz0[:])
            ot1 = work.tile([128, 512], BF16, tag="ot1", bufs=3, name="ot1")
            ots = (ot0, ot1)
            for t in range(4):
                sl = slice(t * 128, (t + 1) * 128)
                nc.gpsimd.partition_all_reduce(
                    m1[:, sl], m1[:, sl], channels=128,
                    reduce_op=bass_isa.ReduceOp.add)
                with nc.allow_low_precision(reason="recip of Z"):
                    nc.vector.reciprocal(m1[:, sl], m1[:, sl])
                nc.vector.tensor_mul(ot1[:, sl], pv1[:, sl], m1[:, sl])
                osb = work.tile([128, D], BF16, tag="osb", bufs=4, name="osb")
                row = out_d.ap()[g * 512 + t * 128:g * 512 + (t + 1) * 128, :]
                for n in range(4):
                    op = pacc.tile([128, 512], F32, tag="acc", bufs=3,
                                   name="op")
                    for hh in (0, 1):
                        nc.tensor.matmul(
                            op[:],
                            ots[hh][:, sl],
                            wo_r[:, hh * D + n * 512:hh * D + (n + 1) * 512],
                            start=(hh == 0), stop=(hh == 1),
                            skip_group_check=True,
                        )
                    if t == 3 and n == 3:
                        # final piece: two half copies on separate engines,
                        # DMA each as soon as it lands
                        nc.vector.tensor_copy(osb[:, n * 512:n * 512 + 256],
                                              op[:, 0:256])
                        nc.sync.dma_start(row[:, n * 512:n * 512 + 256],
                                          osb[:, n * 512:n * 512 + 256])
                        nc.scalar.copy(osb[:, n * 512 + 256:(n + 1) * 512],
                                       op[:, 256:512])
                        nc.sync.dma_start(row[:, n * 512 + 256:(n + 1) * 512],
                                          osb[:, n * 512 + 256:(n + 1) * 512])
                        continue
                    if n % 2 == 1:
                        nc.scalar.copy(osb[:, n * 512:(n + 1) * 512], op[:])
                    else:
                        nc.vector.tensor_copy(osb[:, n * 512:(n + 1) * 512],
                                              op[:])
                    if t == 3:
                        # taper: [128,1024] after n1, [128,512] after n2
                        if n == 1:
                            nc.sync.dma_start(row[:, 0:1024], osb[:, 0:1024])
                        elif n == 2:
                            nc.sync.dma_start(row[:, 1024:1536],
                                              osb[:, 1024:1536])
                if t < 3:
                    nc.sync.dma_start(row, osb[:])

        def op_steps(g, pvz):
            last = g == NG - 1
            while len(pvz) < 3:
                yield False  # wait until tail_steps appended ots
            ots = pvz[2]
            for t in range(4):
                osb = work.tile([128, D], BF16, tag="osb", bufs=4, name="osb")
                for n in range(4):
                    op = (pacc if last else pshared).tile(
                        [128, 512], F32, tag="acc" if last else "sh", bufs=3,
                        name="op")
                    for hh in (0, 1):
                        nc.tensor.matmul(
                            op[:],
                            ots[hh][:, t * 128:(t + 1) * 128],
                            wo_r[:, hh * D + n * 512:hh * D + (n + 1) * 512],
                            start=(hh == 0), stop=(hh == 1),
                            skip_group_check=True,
                        )
                    final = last and t == 3 and n == 3
                    if final:
                        # smallest possible post-PE tail: two half copies on
                        # separate engines, DMA each as soon as it lands
                        nc.vector.tensor_copy(osb[:, n * 512:n * 512 + 256],
                                              op[:, 0:256])
                        nc.sync.dma_start(
                            out_d.ap()[g * 512 + t * 128:g * 512 + (t + 1) * 128,
                                       n * 512:n * 512 + 256],
                            osb[:, n * 512:n * 512 + 256],
                        )
                        nc.scalar.copy(osb[:, n * 512 + 256:(n + 1) * 512],
                                       op[:, 256:512])
                        nc.sync.dma_start(
                            out_d.ap()[g * 512 + t * 128:g * 512 + (t + 1) * 128,
                                       n * 512 + 256:(n + 1) * 512],
                            osb[:, n * 512 + 256:(n + 1) * 512],
                        )
                        yield True
                        continue
                    if last and n % 2 == 1:
                        nc.scalar.copy(osb[:, n * 512:(n + 1) * 512], op[:])
                    else:
                        nc.vector.tensor_copy(osb[:, n * 512:(n + 1) * 512],
                                              op[:])
                    if last:
                        nc.sync.dma_start(
                            out_d.ap()[g * 512 + t * 128:g * 512 + (t + 1) * 128,
                                       n * 512:(n + 1) * 512],
                            osb[:, n * 512:(n + 1) * 512],
                        )
                    yield True
                if not last:
                    nc.sync.dma_start(
                        out_d.ap()[g * 512 + t * 128:g * 512 + (t + 1) * 128,
                                   :],
                        osb[:],
                    )

        # ---- fill machinery ----
        fill_q = deque()

        def pull_fill(n=1):
            # run fill steps until n PE-emitting yields happened (DVE/ACT-only
            # yields don't count, so exp-paced blocks always get real fill)
            got = 0
            while got < n and fill_q:
                try:
                    if next(fill_q[0]):
                        got += 1
                except StopIteration:
                    fill_q.popleft()

        def drain_fill():
            while fill_q:
                pull_fill()

        # ---- attention sweep for group g (emits blocks, pulls fill) ----
        # Block order: both heads' non-diagonal blocks first (h0 then h1),
        # then the diagonal blocks of h0 and h1.  This pushes the kT/v_sb
        # dependency on slice g to the END of the sweep, so slice g's k/v
        # projection passes can serve as fill for sweep g itself.
        def sweep(g, qts):
            nkb = 4 * g + 4
            pvz_out = []
            pv = [None, None]
            master = [None, None]
            quad = [[], []]
            nblk = [0, 0]

            def madd(hh, x_ap, o, w):
                if master[hh] is None:
                    assert o == 0 and w == 512
                    master[hh] = work.tile([128, 512], F32, tag=f"m{hh}",
                                           bufs=3, name=f"m{hh}")
                    nc.vector.tensor_copy(master[hh][:], x_ap)
                elif o == 0 and w == 512:
                    nc.vector.tensor_add(master[hh][:], master[hh][:], x_ap)
                else:
                    nc.vector.tensor_add(master[hh][:, o:o + w],
                                         master[hh][:, o:o + w], x_ap)

            def pv_fold(hh, j, p, diag, o, w, pr):
                # PV + Z-fold for a block, emitted 4 blocks late: the PE is
                # in-order, so the next blocks' sims execute inside this
                # block's exp latency instead of stalling at the PV
                def run():
                    nc.tensor.matmul(
                        pv[hh][:, o:512],
                        v_sb[:, j * 256 + hh * 128:j * 256 + hh * 128 + 128],
                        pr[:, 0:w],
                        start=(nblk[hh] == 0), stop=(nblk[hh] == nkb - 1),
                        skip_group_check=True,
                    )
                    nblk[hh] += 1
                    if not diag:
                        qd = quad[hh]
                        qd.append(pr)
                        if len(qd) == 4:
                            s1 = work.tile([128, 512], BF16, tag="zf",
                                           bufs=10, name="zf")
                            nc.vector.tensor_add(s1[:], qd[0][:], qd[1][:])
                            s2 = work.tile([128, 512], BF16, tag="zf",
                                           bufs=10, name="zf")
                            nc.vector.tensor_add(s2[:], s1[:], qd[2][:])
                            if master[hh] is None:
                                master[hh] = work.tile([128, 512], F32,
                                                       tag=f"m{hh}", bufs=3,
                                                       name=f"m{hh}")
                                nc.vector.tensor_add(master[hh][:], s2[:],
                                                     qd[3][:])
                            else:
                                s3 = work.tile([128, 512], BF16, tag="zf",
                                               bufs=8, name="zf")
                                nc.vector.tensor_add(s3[:], s2[:], qd[3][:])
                                nc.vector.tensor_add(master[hh][:],
                                                     master[hh][:], s3[:])
                            quad[hh] = []
                    else:
                        madd(hh, pr[:, 0:w], o, w)
                return run

            if g == NG - 1:
                # last group: ALL of h0 first (off-diag + diag) so h0's
                # Z/recip chain hides under h1's entire sub-sweep
                order = [(0, j) for j in range(4 * g + 4)] + \
                        [(1, j) for j in range(4 * g + 4)]
            else:
                order = [(hh, j) for hh in range(2) for j in range(4 * g)] + \
                        [(hh, 4 * g + p) for hh in range(2) for p in range(4)]
            pend_blk = deque()
            for (hh, j) in order:
                if pv[hh] is None:
                    pv[hh] = ppv.tile([128, 512], F32, tag="pv", bufs=2,
                                      name=f"pv{hh}")
                p = j - 4 * g
                diag = p >= 0
                o, w = (DO[p], DW[p]) if diag else (0, 512)
                sim = pshared.tile([128, 512], F32, tag="sh", bufs=3,
                                   name="sim")
                nc.tensor.matmul(
                    sim[:, 0:w],
                    kT[hh][:, j * 128:(j + 1) * 128],
                    qts[hh][:, o:512],
                    start=True, stop=True, skip_group_check=True,
                )
                pr = work.tile([128, 512], BF16, tag="pr", bufs=14,
                               name="pr")
                nc.scalar.activation(pr[:, 0:w], sim[:, 0:w], Exp,
                                     scale=SCALE)
                if diag:
                    nc.vector.tensor_mul(pr[:, 0:w], pr[:, 0:w], masks[p])
                pull_fill(2 if diag else 1)
                if len(pend_blk) >= 4:
                    pend_blk.popleft()()
                pend_blk.append(pv_fold(hh, j, p, diag, o, w, pr))
                yield
                if nblk[0] == nkb and master[0] is not None and \
                        len(pvz_out) == 0:
                    # h0 finished all its blocks: allreduce+recip now (Pool
                    # + DVE, zero PE) while h1's sub-sweep continues
                    pull_fill(2)
                    finish_head(0, master[0])
                    pvz_out.append((pv[0], master[0]))
            while pend_blk:  # flush the trailing blocks' pv + fold
                pend_blk.popleft()()
            assert not quad[0] and not quad[1]
            pvz_out.append((pv[1], master[1]))
            return pvz_out

        # ================= main program =================
        xs0, cos0, sin0 = issue_x(0, interleave_w=True)
        nc.sync.dma_start(wo_r[:, 0:D], wo_d.ap()[0:128, :])
        nc.sync.dma_start(wo_r[:, D:2 * D], wo_d.ap()[128:256, :])

        qts_all = {}

        def new_qts():
            return [work.tile([128, 512], BF16, tag=f"qt{h}", bufs=3,
                              name=f"qt{h}") for h in range(2)]

        # slice 0 runs un-filled (nothing to overlap with yet)
        qts_all[0] = new_qts()
        for _ in slice_steps(0, xs0, cos0, sin0, qts_all[0], startup=True):
            pass

        for g in range(NG):
            if g + 1 < NG:
                xs_n, cos_n, sin_n = issue_x(g + 1)
                qts_all[g + 1] = new_qts()
                if g + 1 == NG - 1:
                    # last slice: q and k0 precede sweep 7 (h0's diag blocks
                    # come at blocks 29-32 now); v fills sweep 7 first (its
                    # copies land at pulls ~17/33, before the h0-diag PV
                    # flush), then k1 (rope at ~pull 50, before h1's diags),
                    # then tail6/op6 cover the rest
                    fill_q.append(slice_steps(g + 1, xs_n, cos_n, sin_n,
                                              qts_all[g + 1],
                                              pairs=PAIRS[:2]))
                    v7 = slice_steps(g + 1, xs_n, cos_n, sin_n,
                                     qts_all[g + 1], pairs=PAIRS[2:])
                else:
                    fill_q.append(
                        slice_steps(g + 1, xs_n, cos_n, sin_n,
                                    qts_all[g + 1]))
            # run the sweep (pulls fill: [tail g-1, op g-1, proj g+1])
            import os
            if os.environ.get("DEBUG_NO_FILL") == "1":
                drain_fill()
            sw = sweep(g, qts_all[g])
            pvz = None
            try:
                while True:
                    next(sw)
            except StopIteration as e:
                pvz = e.value
            # everything queued must land before the next sweep's sims
            drain_fill()
            if g == NG - 1:
                last_tail_op(g, pvz)
                if _dbg:
                    nc.sync.dma_start(dbg_k0.ap(), kT[0][:])
                    nc.sync.dma_start(dbg_k1.ap(), kT[1][:])
                    nc.sync.dma_start(dbg_v.ap(), v_sb[:])
            else:
                if g + 1 == NG - 1:
                    fill_q.append(v7)
                fill_q.append(tail_steps(g, pvz))
                fill_q.append(op_steps(g, pvz))

    nc.compile()
    return nc


def _host_tables(S: int):
    inv = 1.0 / (ROPE_BASE ** (np.arange(0, DH, 2, dtype=np.float64) / DH))
    t = np.arange(S, dtype=np.float64)
    fr = np.outer(t, inv)  # [S, 64]
    cos = np.repeat(np.cos(fr), 2, axis=1)
    sin = np.repeat(np.sin(fr), 2, axis=1)
    sinA = sin.T.copy()           # [128 dh, S]
    sinA[0::2, :] *= -1.0         # negate even-dh rows (rotate_half sign)
    cs = np.concatenate([cos.T, sinA], axis=1).astype(BF)  # [128, 2S]

    # diagonal-block causal masks: tile col c (query 128p+c) vs key partition
    cb = np.zeros((128, 2048), np.float32)
    part = np.arange(128)[:, None]
    for p in range(4):
        w = DW[p]
        c = np.arange(w)[None, :]
        cb[:, p * 512:p * 512 + w] = (c >= part).astype(np.float32)
    cb = cb.astype(BF)
    return cs, cb


def kernel(x, mask, wq, wk, wv, wo):
    x = np.asarray(x, dtype=np.float32)
    wq = np.asarray(wq, dtype=np.float32)
    wk = np.asarray(wk, dtype=np.float32)
    wv = np.asarray(wv, dtype=np.float32)
    wo = np.asarray(wo, dtype=np.float32)
    S = x.shape[0]

    if S not in _BUILD_CACHE:
        _BUILD_CACHE[S] = _build(S)
    nc = _BUILD_CACHE[S]

    cs, cb = _host_tables(S)
    xT = np.ascontiguousarray(x.T.astype(BF))

    in_maps = []
    for c in range(NCORES):
        hsl = slice(c * HPC * DH, (c + 1) * HPC * DH)
        wqT = wq[hsl].T.reshape(D, 2, DH)
        wkT = wk[hsl].T.reshape(D, 2, DH)
        wvT = wv[hsl].T.reshape(D, 2, DH)
        wqkvT = np.concatenate(
            [wqT[:, 0], wqT[:, 1], wkT[:, 0], wkT[:, 1], wvT[:, 0], wvT[:, 1]],
            axis=1,
        ).astype(BF)
        woT = np.ascontiguousarray(wo[:, hsl].T.astype(BF))
        in_maps.append(
            {
                "xT": xT,
                "wqkvT": np.ascontiguousarray(wqkvT),
                "woT": woT,
                "cs": cs,
                "cb": cb,
            }
        )

    res = run_bass_kernel_spmd(
        nc, in_maps, core_ids=list(range(NCORES)), trace=TRACE
    )
    global LAST_RESULT
    LAST_RESULT = res
    out = np.zeros((S, D), np.float32)
    for r in res.results:
        out += np.asarray(r["outp"], dtype=np.float32)
    return out


# revision 31
# speedup vs baseline: 1.0020x; 1.0020x over previous
"""Trainium2 Bass kernel for nn_Attention_48000554500172.

16-head causal attention with RoPE (S=4096, D=2048, H=16, DH=128), sharded
over heads across 8 NeuronCores (2 heads/core, tensor parallel). Each core
computes its heads' QKV projections, RoPE, causal softmax attention and the
partial output projection; the 8 partial [S, D] outputs are summed on host
(the all-reduce of the sharding hint).

v3 design (v2 was 373us; TimelineSim ~341us):
- All matmul inputs bf16 (1 cycle/row on the PE at any moving size); PSUM
  accumulation fp32; output shipped bf16, partials summed f32 on host.
- RoPE entirely off the PE: rotate_half is a DVE stream_shuffle (pairwise
  partition swap within 32-blocks) and the sin table has its even-dh rows
  negated on host, so q' = q*cos + shuf(q)*sinA with two 4x-rate
  scalar_tensor_tensor muls + one add. (v2 used a PE permutation matmul per
  rope step + a PSUM-read mul.)
- V is produced directly in [keys, dh] layout: the v projection uses the x
  chunk as the STATIONARY operand (out partitions = s-tile) in 4 column
  quadrants, accumulating over d in PSUM; one strided copy lands v_sb.
  Kills the per-slice PE transposes and the extra drain.
- Softmax denominators: probs tiles fold on the DVE (bf16 quad trees via
  4x scalar_tensor_tensor + an f32 running master per head/group; diagonal
  blocks fold in-place at their causal widths). The 128-partition sum runs
  on the idle GPSIMD (partition_all_reduce, in-place), the reciprocal is a
  full-width DVE op, and ot = pv * (1/Z) reads it directly - no PE
  ones-matmuls, no PE broadcast matmul, no ACT bcs copies.
- Causal masking of diagonal blocks via precomputed 0/1 bf16 mask tiles
  applied with 4x-rate in-place scalar_tensor_tensor muls.
- Phase interleaving: the attention sweep is exp-paced on the ACT engine
  (612ns/block vs 426ns of PE sim+PV work), so the projection matmuls of
  slice g+1 and the tail+output projection of group g-1 are emitted as
  fill between the sweep blocks of group g, one matmul per sweep block
  (finer than v2's two, so fill lasts the whole last sweep). The sweep is
  software-pipelined 4 blocks deep (PV + Z-fold emitted 4 blocks after
  sim/exp).
- Startup: the first w and x chunks are split so the first matmul's
  operands arrive in ~2.5us; slice 0's first projection group is 6
  accumulators wide to cover the arriving stream.
- Endgame: the last group's output DMAs go out per 512-column block (the
  final block split in two) so the post-PE tail is one small copy + DMA.
"""
import math
import numpy as np
import ml_dtypes
from collections import deque
from contextlib import ExitStack

import concourse.bass as bass
import concourse.tile as tile
from concourse import bacc, mybir, bass_isa
from concourse.bass_utils import run_bass_kernel_spmd

D, H, DH = 2048, 16, 128
NCORES = 8
HPC = H // NCORES  # 2 heads per core
ROPE_BASE = 10000.0
SCALE = 1.0 / math.sqrt(DH)
F32 = mybir.dt.float32
BF16 = mybir.dt.bfloat16
Exp = mybir.ActivationFunctionType.Exp
Mult = mybir.AluOpType.mult
Add = mybir.AluOpType.add
BF = ml_dtypes.bfloat16

_BUILD_CACHE: dict = {}
TRACE = False
LAST_RESULT = None

# diagonal key-block p: query cols [128p, 512) are unmasked
DW = (512, 384, 256, 128)
DO = (0, 128, 256, 384)

# pairwise swap within 32-partition blocks (rotate_half partner)
SWAP_MASK = [m ^ 1 for m in range(32)]


def _build(S: int):
    assert S % 512 == 0
    NG = S // 512   # 8 query groups == s-slices
    ND = D // 128   # 16 contraction tiles
    NB = S // 128   # 32 key blocks

    nc = bacc.Bacc("TRN2", target_bir_lowering=False, debug=False)

    xT_d = nc.dram_tensor("xT", [D, S], BF16, kind="ExternalInput")
    w_d = nc.dram_tensor("wqkvT", [D, 768], BF16, kind="ExternalInput")
    wo_d = nc.dram_tensor("woT", [2 * DH, D], BF16, kind="ExternalInput")
    cs_d = nc.dram_tensor("cs", [128, 2 * S], BF16, kind="ExternalInput")
    cb_d = nc.dram_tensor("cb", [128, 2048], BF16, kind="ExternalInput")
    out_d = nc.dram_tensor("outp", [S, D], BF16, kind="ExternalOutput")

    with tile.TileContext(nc) as tc, ExitStack() as ctx:
        persist = ctx.enter_context(tc.tile_pool(name="persist", bufs=1))
        work = ctx.enter_context(tc.tile_pool(name="work", bufs=2))
        pacc = ctx.enter_context(tc.tile_pool(name="pacc", bufs=3, space="PSUM"))
        pshared = ctx.enter_context(tc.tile_pool(name="psh", bufs=3, space="PSUM"))
        ppv = ctx.enter_context(tc.tile_pool(name="ppv", bufs=2, space="PSUM"))

        # ---- persistent tiles ----
        w_r = persist.tile([128, ND * 768], BF16, tag="w", name="wr")
        wo_r = persist.tile([128, 2 * D], BF16, tag="wo", name="wor")
        kT = [persist.tile([128, S], BF16, tag=f"kT{h}", name=f"kT{h}")
              for h in range(2)]
        v_sb = persist.tile([128, NB * 256], BF16, tag="v", name="vsb")
        cb_t = persist.tile([128, 2048], BF16, tag="cb", name="cbt")
        masks = [cb_t[:, p * 512:p * 512 + DW[p]] for p in range(4)]

        # ---- DMA issue helpers ----
        def issue_x(g, interleave_w=False):
            """DMA x chunks (+cos/sin) for slice g; optionally interleave the
            16 w chunks + cb in d-consumption order (startup)."""
            xs = []
            for dd in range(ND // 2):
                t = work.tile([128, 1024], BF16, tag="xs", bufs=20, name="xs")
                src = xT_d.ap()[dd * 256:(dd + 1) * 256, g * 512:(g + 1) * 512]
                if interleave_w and dd == 0:
                    # split first pieces so the first matmul starts ~2.5us:
                    # w jt0-1 of d0..d127, then x d-step 0, then the rest
                    nc.sync.dma_start(w_r[:, 0:256], w_d.ap()[0:128, 0:256])
                    nc.sync.dma_start(
                        t[:, 0:512], src[0:128, :])
                    nc.sync.dma_start(w_r[:, 256:768], w_d.ap()[0:128, 256:768])
                    nc.sync.dma_start(
                        t[:, 512:1024], src[128:256, :])
                    nc.sync.dma_start(
                        w_r[:, 768:1536], w_d.ap()[128:256, :])
                    xs.append(t)
                    continue
                nc.sync.dma_start(
                    t[:].rearrange("b (a c) -> b a c", a=2),
                    src.rearrange("(a b) c -> b a c", a=2),
                )
                xs.append(t)
                if interleave_w:
                    # two w chunks in one strided DMA, consumption order
                    wdst = w_r[:, dd * 1536:(dd + 1) * 1536]
                    wsrc = w_d.ap()[dd * 256:(dd + 1) * 256, :]
                    nc.sync.dma_start(
                        wdst.rearrange("b (a c) -> b a c", a=2),
                        wsrc.rearrange("(a b) c -> b a c", a=2),
                    )
                    if dd == 3:
                        # masks: needed first at diag-block time (~13us)
                        nc.sync.dma_start(cb_t[:], cb_d.ap())
            cos = work.tile([128, 512], BF16, tag="cos", bufs=3, name="cos")
            sin = work.tile([128, 512], BF16, tag="sin", bufs=3, name="sin")
            nc.sync.dma_start(cos[:], cs_d.ap()[:, g * 512:(g + 1) * 512])
            nc.sync.dma_start(sin[:], cs_d.ap()[:, S + g * 512:S + (g + 1) * 512])
            return xs, cos, sin

        # ---- slice compute (projections + rope + v), as a fill generator ----
        # groups: ("q", jts), ("k", jts) are 512-wide accumulator passes;
        # ("v",) runs both heads' quad passes (stationary = x chunk).
        # Yields True when the step emitted PE work, False for DVE/ACT-only
        # steps; pull_fill only counts True yields so exp-paced sweep blocks
        # always receive real PE fill.
        PAIRS = (("q", (0, 1)), ("k", (2, 3)), ("v",))

        def slice_steps(g, xs, cos, sin, qts, pairs=PAIRS, startup=False):
            pend = []  # deferred post-steps, run with lag

            def rope_step(kind, hh, t_in):
                def run():
                    shf = work.tile([128, 512], BF16, tag="shf", bufs=3,
                                    name="shf")
                    nc.vector.stream_shuffle(shf[:], t_in[:], SWAP_MASK)
                    t1 = work.tile([128, 512], BF16, tag="t1", bufs=3,
                                   name="t1")
                    nc.vector.tensor_mul(t1[:], t_in[:], cos[:])
                    t2 = work.tile([128, 512], BF16, tag="t2", bufs=3,
                                   name="t2")
                    nc.vector.tensor_mul(t2[:], shf[:], sin[:])
                    if kind == "q":
                        nc.vector.tensor_add(qts[hh][:], t1[:], t2[:])
                    else:
                        nc.vector.tensor_add(
                            kT[hh][:, g * 512:(g + 1) * 512], t1[:], t2[:])
                return run

            def v_copy_step(hh, vq):
                def run():
                    dst = v_sb[:].rearrange("p (b x) -> p b x", x=256)[
                        :, 4 * g:4 * g + 4, hh * 128:(hh + 1) * 128]
                    src = vq[:].rearrange("p (b x) -> p b x", x=128)
                    nc.scalar.copy(dst, src)
                return run

            # startup: one 6-wide d-major pass using every idle PSUM bank
            # (q0,q1 pacc; k0,k1 psh+psh; vq0 psh... use pacc3+psh2+ppv1)
            if startup:
                POOL6 = ((pacc, "acc", 3), (pacc, "acc", 3),
                         (pacc, "acc", 3), (pshared, "sh", 3),
                         (pshared, "sh", 3), (ppv, "pv", 2))
                accs = [POOL6[i][0].tile([128, 512], F32, tag=POOL6[i][1],
                                         bufs=POOL6[i][2], name="acc6")
                        for i in range(6)]
                qk, vqs = accs[:4], accs[4:]
                for d in range(ND):
                    xr = xs[d // 2][:, (d % 2) * 512:(d % 2) * 512 + 512]
                    for jt in range(4):
                        nc.tensor.matmul(
                            qk[jt][:],
                            w_r[:, d * 768 + jt * 128:d * 768 + (jt + 1) * 128],
                            xr,
                            start=(d == 0), stop=(d == ND - 1),
                            skip_group_check=True,
                        )
                        yield True
                    for hh in range(2):
                        for q in range(4):
                            # ONE start per psum bank row: start_tensor_calc
                            # arms pending-zero for the whole 2KB region, so
                            # later quads' first writes overwrite-then-clear
                            nc.tensor.matmul(
                                vqs[hh][:, q * 128:(q + 1) * 128],
                                xr[:, q * 128:(q + 1) * 128],
                                w_r[:, d * 768 + (4 + hh) * 128:
                                    d * 768 + (5 + hh) * 128],
                                start=(d == 0 and q == 0),
                                stop=(d == ND - 1 and q == 3),
                                skip_group_check=True,
                            )
                        yield True
                KINDS = ("q", "q", "k", "k")
                for jt in range(4):
                    hh = jt % 2
                    t_in = work.tile([128, 512], BF16, tag="tin", bufs=6,
                                     name="tin")
                    nc.scalar.copy(t_in[:, 0:256], qk[jt][:, 0:256])
                    nc.vector.tensor_copy(t_in[:, 256:512],
                                          qk[jt][:, 256:512])
                    pend.append(rope_step(KINDS[jt], hh, t_in))
                for hh in range(2):
                    pend.append(v_copy_step(hh, vqs[hh]))
                yield False
                while pend:
                    pend.pop(0)()
                    yield False
                return

            for grp in pairs:
                kind = grp[0]
                if kind == "v":
                    for hh in range(2):
                        vq = pacc.tile([128, 512], F32, tag="acc", bufs=3,
                                       name="vq")
                        for d in range(ND):
                            xr = xs[d // 2][:, (d % 2) * 512:
                                            (d % 2) * 512 + 512]
                            for q in range(4):
                                # ONE start per psum bank row (see startup)
                                nc.tensor.matmul(
                                    vq[:, q * 128:(q + 1) * 128],
                                    xr[:, q * 128:(q + 1) * 128],
                                    w_r[:, d * 768 + (4 + hh) * 128:
                                        d * 768 + (5 + hh) * 128],
                                    start=(d == 0 and q == 0),
                                    stop=(d == ND - 1 and q == 3),
                                    skip_group_check=True,
                                )
                            yield True
                        v_copy_step(hh, vq)()
                    continue
                jts = grp[1]
                acc = [pacc.tile([128, 512], F32, tag="acc", bufs=3,
                                 name="acc") for _ in jts]
                for d in range(ND):
                    xr = xs[d // 2][:, (d % 2) * 512:(d % 2) * 512 + 512]
                    for i, jt in enumerate(jts):
                        nc.tensor.matmul(
                            acc[i][:],
                            w_r[:, d * 768 + jt * 128:d * 768 + (jt + 1) * 128],
                            xr,
                            start=(d == 0), stop=(d == ND - 1),
                            skip_group_check=True,
                        )
                        yield True
                # drain the accs to SBUF (ACT + DVE in parallel); defer the
                # rope halves for later pulls
                for i, jt in enumerate(jts):
                    hh = jt % 2
                    t_in = work.tile([128, 512], BF16, tag="tin", bufs=6,
                                     name="tin")
                    nc.scalar.copy(t_in[:, 0:256], acc[i][:, 0:256])
                    nc.vector.tensor_copy(t_in[:, 256:512],
                                          acc[i][:, 256:512])
                    pend.append(rope_step(kind, hh, t_in))
                yield False
                while pend:
                    pend.pop(0)()
                    yield False
            while pend:
                pend.pop(0)()
                yield False

        # ---- group tail (Z allreduce/recip/ot) + output projection ----
        def finish_head(hh, master, half=None):
            """In-place: master <- 1/sum_partitions(master). half: 0/1 for
            column-half recip (last group's critical chain)."""
            if half is None:
                nc.gpsimd.partition_all_reduce(
                    master[:], master[:], channels=128,
                    reduce_op=bass_isa.ReduceOp.add)
                with nc.allow_low_precision(reason="recip of Z"):
                    nc.vector.reciprocal(master[:], master[:])
            else:
                sl = slice(half * 256, half * 256 + 256)
                with nc.allow_low_precision(reason="recip of Z"):
                    nc.vector.reciprocal(master[:, sl], master[:, sl])

        def tail_steps(g, pvz):
            last = g == NG - 1
            ots = []
            for hh in range(2):
                if hh == 0:
                    pv, rz = pvz[0]  # recip already done inline in sweep
                    ot = work.tile([128, 512], BF16, tag="ot0", bufs=3,
                                   name="ot0")
                    nc.vector.tensor_mul(ot[:], pv[:], rz[:])
                    ots.append(ot)
                    yield False
                    continue
                pv, master = pvz[1]
                if last:
                    # final critical chain: allreduce -> recip/ot per column
                    # half so the output projection starts on the first half
                    nc.gpsimd.partition_all_reduce(
                        master[:], master[:], channels=128,
                        reduce_op=bass_isa.ReduceOp.add)
                    ot = work.tile([128, 512], BF16, tag="ot1", bufs=3,
                                   name="ot1")
                    for half in range(2):
                        sl = slice(half * 256, half * 256 + 256)
                        finish_head(hh, master, half=half)
                        nc.vector.tensor_mul(ot[:, sl], pv[:, sl],
                                             master[:, sl])
                    ots.append(ot)
                    yield
                    continue
                finish_head(hh, master)
                yield False
                ot = work.tile([128, 512], BF16, tag="ot1", bufs=3,
                               name="ot1")
                nc.vector.tensor_mul(ot[:], pv[:], master[:])
                ots.append(ot)
                yield False
            pvz.append(ots)  # hand ots to op_steps via shared list

        def last_tail_op(g, pvz):
            pv0, rz0 = pvz[0]   # h0: 1/Z computed inline in the sweep
            pv1, m1 = pvz[1]
            ot0 = work.tile([128, 512], BF16, tag="ot0", bufs=3, name="ot0")
            nc.vector.tensor_mul(ot0[:], pv0[:], rz0[:])
            ot1 = work.tile([128, 512], BF16, tag="ot1", bufs=3, name="ot1")
            ots = (ot0, ot1)
            for t in range(4):
                sl = slice(t * 128, (t + 1) * 128)
                nc.gpsimd.partition_all_reduce(
                    m1[:, sl], m1[:, sl], channels=128,
                    reduce_op=bass_isa.ReduceOp.add)
                with nc.allow_low_precision(reason="recip of Z"):
                    nc.vector.reciprocal(m1[:, sl], m1[:, sl])
                nc.vector.tensor_mul(ot1[:, sl], pv1[:, sl], m1[:, sl])
                osb = work.tile([128, D], BF16, tag="osb", bufs=4, name="osb")
                row = out_d.ap()[g * 512 + t * 128:g * 512 + (t + 1) * 128, :]
                for n in range(4):
                    op = pacc.tile([128, 512], F32, tag="acc", bufs=3,
                                   name="op")
                    for hh in (0, 1):
                        nc.tensor.matmul(
                            op[:],
                            ots[hh][:, sl],
                            wo_r[:, hh * D + n * 512:hh * D + (n + 1) * 512],
                            start=(hh == 0), stop=(hh == 1),
                            skip_group_check=True,
                        )
                    if t == 3 and n == 3:
                        # final piece: two half copies on separate engines,
                        # DMA each as soon as it lands
                        nc.vector.tensor_copy(osb[:, n * 512:n * 512 + 256],
                                              op[:, 0:256])
                        nc.sync.dma_start(row[:, n * 512:n * 512 + 256],
                                          osb[:, n * 512:n * 512 + 256])
                        nc.scalar.copy(osb[:, n * 512 + 256:(n + 1) * 512],
                                       op[:, 256:512])
                        nc.sync.dma_start(row[:, n * 512 + 256:(n + 1) * 512],
                                          osb[:, n * 512 + 256:(n + 1) * 512])
                        continue
                    if n % 2 == 1:
                        nc.scalar.copy(osb[:, n * 512:(n + 1) * 512], op[:])
                    else:
                        nc.vector.tensor_copy(osb[:, n * 512:(n + 1) * 512],
                                              op[:])
                    if t == 3:
                        # taper: [128,1024] after n1, [128,512] after n2
                        if n == 1:
                            nc.sync.dma_start(row[:, 0:1024], osb[:, 0:1024])
                        elif n == 2:
                            nc.sync.dma_start(row[:, 1024:1536],
                                              osb[:, 1024:1536])
                if t < 3:
                    nc.sync.dma_start(row, osb[:])

        def op_steps(g, pvz):
            last = g == NG - 1
            while len(pvz) < 3:
                yield False  # wait until tail_steps appended ots
            ots = pvz[2]
            for t in range(4):
                osb = work.tile([128, D], BF16, tag="osb", bufs=4, name="osb")
                for n in range(4):
                    op = (pacc if last else pshared).tile(
                        [128, 512], F32, tag="acc" if last else "sh", bufs=3,
                        name="op")
                    for hh in (0, 1):
                        nc.tensor.matmul(
                            op[:],
                            ots[hh][:, t * 128:(t + 1) * 128],
                            wo_r[:, hh * D + n * 512:hh * D + (n + 1) * 512],
                            start=(hh == 0), stop=(hh == 1),
                            skip_group_check=True,
                        )
                    final = last and t == 3 and n == 3
                    if final:
                        # smallest possible post-PE tail: two half copies on
                        # separate engines, DMA each as soon as it lands
                        nc.vector.tensor_copy(osb[:, n * 512:n * 512 + 256],
                                              op[:, 0:256])
                        nc.sync.dma_start(
                            out_d.ap()[g * 512 + t * 128:g * 512 + (t + 1) * 128,
                                       n * 512:n * 512 + 256],
                            osb[:, n * 512:n * 512 + 256],
                        )
                        nc.scalar.copy(osb[:, n * 512 + 256:(n + 1) * 512],
                                       op[:, 256:512])
                        nc.sync.dma_start(
                            out_d.ap()[g * 512 + t * 128:g * 512 + (t + 1) * 128,
                                       n * 512 + 256:(n + 1) * 512],
                            osb[:, n * 512 + 256:(n + 1) * 512],
                        )
                        yield True
                        continue
                    if last and n % 2 == 1:
                        nc.scalar.copy(osb[:, n * 512:(n + 1) * 512], op[:])
                    else:
                        nc.vector.tensor_copy(osb[:, n * 512:(n + 1) * 512],
                                              op[:])
                    if last:
                        nc.sync.dma_start(
                            out_d.ap()[g * 512 + t * 128:g * 512 + (t + 1) * 128,
                                       n * 512:(n + 1) * 512],
                            osb[:, n * 512:(n + 1) * 512],
                        )
                    yield True
                if not last:
                    nc.sync.dma_start(
                        out_d.ap()[g * 512 + t * 128:g * 512 + (t + 1) * 128,
                                   :],
                        osb[:],
                    )

        # ---- fill machinery ----
        fill_q = deque()

        def pull_fill(n=1):
            # run fill steps until n PE-emitting yields happened (DVE/ACT-only
            # yields don't count, so exp-paced blocks always get real fill)
            got = 0
            while got < n and fill_q:
                try:
                    if next(fill_q[0]):
                        got += 1
                except StopIteration:
                    fill_q.popleft()

        def drain_fill():
            while fill_q:
                pull_fill()

        # ---- attention sweep for group g (emits blocks, pulls fill) ----
        # Block order: both heads' non-diagonal blocks first (h0 then h1),
        # then the diagonal blocks of h0 and h1.  This pushes the kT/v_sb
        # dependency on slice g to the END of the sweep, so slice g's k/v
        # projection passes can serve as fill for sweep g itself.
        def sweep(g, qts):
            nkb = 4 * g + 4
            pvz_out = []
            pv = [None, None]
            master = [None, None]
            quad = [[], []]
            nblk = [0, 0]

            def madd(hh, x_ap, o, w):
                if master[hh] is None:
                    assert o == 0 and w == 512
                    master[hh] = work.tile([128, 512], F32, tag=f"m{hh}",
                                           bufs=3, name=f"m{hh}")
                    nc.vector.tensor_copy(master[hh][:], x_ap)
                elif o == 0 and w == 512:
                    nc.vector.tensor_add(master[hh][:], master[hh][:], x_ap)
                else:
                    nc.vector.tensor_add(master[hh][:, o:o + w],
                                         master[hh][:, o:o + w], x_ap)

            def pv_fold(hh, j, p, diag, o, w, pr):
                # PV + Z-fold for a block, emitted 4 blocks late: the PE is
                # in-order, so the next blocks' sims execute inside this
                # block's exp latency instead of stalling at the PV
                def run():
                    nc.tensor.matmul(
                        pv[hh][:, o:512],
                        v_sb[:, j * 256 + hh * 128:j * 256 + hh * 128 + 128],
                        pr[:, 0:w],
                        start=(nblk[hh] == 0), stop=(nblk[hh] == nkb - 1),
                        skip_group_check=True,
                    )
                    nblk[hh] += 1
                    if not diag:
                        qd = quad[hh]
                        qd.append(pr)
                        if len(qd) == 4:
                            s1 = work.tile([128, 512], BF16, tag="zf",
                                           bufs=10, name="zf")
                            nc.vector.tensor_add(s1[:], qd[0][:], qd[1][:])
                            s2 = work.tile([128, 512], BF16, tag="zf",
                                           bufs=10, name="zf")
                            nc.vector.tensor_add(s2[:], s1[:], qd[2][:])
                            if master[hh] is None:
                                master[hh] = work.tile([128, 512], F32,
                                                       tag=f"m{hh}", bufs=3,
                                                       name=f"m{hh}")
                                nc.vector.tensor_add(master[hh][:], s2[:],
                                                     qd[3][:])
                            else:
                                s3 = work.tile([128, 512], BF16, tag="zf",
                                               bufs=8, name="zf")
                                nc.vector.tensor_add(s3[:], s2[:], qd[3][:])
                                nc.vector.tensor_add(master[hh][:],
                                                     master[hh][:], s3[:])
                            quad[hh] = []
                    else:
                        madd(hh, pr[:, 0:w], o, w)
                return run

            if g == NG - 1:
                # last group: ALL of h0 first (off-diag + diag) so h0's
                # Z/recip chain hides under h1's entire sub-sweep
                order = [(0, j) for j in range(4 * g + 4)] + \
                        [(1, j) for j in range(4 * g + 4)]
            else:
                order = [(hh, j) for hh in range(2) for j in range(4 * g)] + \
                        [(hh, 4 * g + p) for hh in range(2) for p in range(4)]
            pend_blk = deque()
            for (hh, j) in order:
                if pv[hh] is None:
                    pv[hh] = ppv.tile([128, 512], F32, tag="pv", bufs=2,
                                      name=f"pv{hh}")
                p = j - 4 * g
                diag = p >= 0
                o, w = (DO[p], DW[p]) if diag else (0, 512)
                sim = pshared.tile([128, 512], F32, tag="sh", bufs=3,
                                   name="sim")
                nc.tensor.matmul(
                    sim[:, 0:w],
                    kT[hh][:, j * 128:(j + 1) * 128],
                    qts[hh][:, o:512],
                    start=True, stop=True, skip_group_check=True,
                )
                pr = work.tile([128, 512], BF16, tag="pr", bufs=14,
                               name="pr")
                nc.scalar.activation(pr[:, 0:w], sim[:, 0:w], Exp,
                                     scale=SCALE)
                if diag:
                    nc.vector.tensor_mul(pr[:, 0:w], pr[:, 0:w], masks[p])
                pull_fill(2 if diag else 1)
                if len(pend_blk) >= 4:
                    pend_blk.popleft()()
                pend_blk.append(pv_fold(hh, j, p, diag, o, w, pr))
                yield
                if nblk[0] == nkb and master[0] is not None and \
                        len(pvz_out) == 0:
                    # h0 finished all its blocks: allreduce+recip now (Pool
                    # + DVE, zero PE) while h1's sub-sweep continues
                    pull_fill(2)
                    finish_head(0, master[0])
                    pvz_out.append((pv[0], master[0]))
            while pend_blk:  # flush the trailing blocks' pv + fold
                pend_blk.popleft()()
            assert not quad[0] and not quad[1]
            pvz_out.append((pv[1], master[1]))
            return pvz_out

        # ================= main program =================
        xs0, cos0, sin0 = issue_x(0, interleave_w=True)
        nc.sync.dma_start(wo_r[:, 0:D], wo_d.ap()[0:128, :])
        nc.sync.dma_start(wo_r[:, D:2 * D], wo_d.ap()[128:256, :])

        qts_all = {}

        def new_qts():
            return [work.tile([128, 512], BF16, tag=f"qt{h}", bufs=3,
                              name=f"qt{h}") for h in range(2)]

        # slice 0 runs un-filled (nothing to overlap with yet)
        qts_all[0] = new_qts()
        for _ in slice_steps(0, xs0, cos0, sin0, qts_all[0], startup=True):
            pass

        for g in range(NG):
            if g + 1 < NG:
                xs_n, cos_n, sin_n = issue_x(g + 1)
                qts_all[g + 1] = new_qts()
                if g + 1 == NG - 1:
                    # last slice: q and k0 precede sweep 7 (h0's diag blocks
                    # come at blocks 29-32 now); v fills sweep 7 first (its
                    # copies land at pulls ~17/33, before the h0-diag PV
                    # flush), then k1 (rope at ~pull 50, before h1's diags),
                    # then tail6/op6 cover the rest
                    fill_q.append(slice_steps(g + 1, xs_n, cos_n, sin_n,
                                              qts_all[g + 1],
                                              pairs=PAIRS[:2]))
                    v7 = slice_steps(g + 1, xs_n, cos_n, sin_n,
                                     qts_all[g + 1], pairs=PAIRS[2:])
                else:
                    fill_q.append(
                        slice_steps(g + 1, xs_n, cos_n, sin_n,
                                    qts_all[g + 1]))
            # run the sweep (pulls fill: [tail g-1, op g-1, proj g+1])
            sw = sweep(g, qts_all[g])
            pvz = None
            try:
                while True:
                    next(sw)
            except StopIteration as e:
                pvz = e.value
            # everything queued must land before the next sweep's sims
            drain_fill()
            if g == NG - 1:
                last_tail_op(g, pvz)
            else:
                if g + 1 == NG - 1:
                    fill_q.append(v7)
                fill_q.append(tail_steps(g, pvz))
                fill_q.append(op_steps(g, pvz))

    nc.compile()
    return nc


def _host_tables(S: int):
    inv = 1.0 / (ROPE_BASE ** (np.arange(0, DH, 2, dtype=np.float64) / DH))
    t = np.arange(S, dtype=np.float64)
    fr = np.outer(t, inv)  # [S, 64]
    cos = np.repeat(np.cos(fr), 2, axis=1)
    sin = np.repeat(np.sin(fr), 2, axis=1)
    sinA = sin.T.copy()           # [128 dh, S]
    sinA[0::2, :] *= -1.0         # negate even-dh rows (rotate_half sign)
    cs = np.concatenate([cos.T, sinA], axis=1).astype(BF)  # [128, 2S]

    # diagonal-block causal masks: tile col c (query 128p+c) vs key partition
    cb = np.zeros((128, 2048), np.float32)
    part = np.arange(128)[:, None]
    for p in range(4):
        w = DW[p]
        c = np.arange(w)[None, :]
        cb[:, p * 512:p * 512 + w] = (c >= part).astype(np.float32)
    cb = cb.astype(BF)
    return cs, cb


def kernel(x, mask, wq, wk, wv, wo):
    x = np.asarray(x, dtype=np.float32)
    wq = np.asarray(wq, dtype=np.float32)
    wk = np.asarray(wk, dtype=np.float32)
    wv = np.asarray(wv, dtype=np.float32)
    wo = np.asarray(wo, dtype=np.float32)
    S = x.shape[0]

    if S not in _BUILD_CACHE:
        _BUILD_CACHE[S] = _build(S)
    nc = _BUILD_CACHE[S]

    cs, cb = _host_tables(S)
    xT = np.ascontiguousarray(x.T.astype(BF))

    in_maps = []
    for c in range(NCORES):
        hsl = slice(c * HPC * DH, (c + 1) * HPC * DH)
        wqT = wq[hsl].T.reshape(D, 2, DH)
        wkT = wk[hsl].T.reshape(D, 2, DH)
        wvT = wv[hsl].T.reshape(D, 2, DH)
        wqkvT = np.concatenate(
            [wqT[:, 0], wqT[:, 1], wkT[:, 0], wkT[:, 1], wvT[:, 0], wvT[:, 1]],
            axis=1,
        ).astype(BF)
        woT = np.ascontiguousarray(wo[:, hsl].T.astype(BF))
        in_maps.append(
            {
                "xT": xT,
                "wqkvT": np.ascontiguousarray(wqkvT),
                "woT": woT,
                "cs": cs,
                "cb": cb,
            }
        )

    res = run_bass_kernel_spmd(
        nc, in_maps, core_ids=list(range(NCORES)), trace=TRACE
    )
    global LAST_RESULT
    LAST_RESULT = res
    out = np.zeros((S, D), np.float32)
    for r in res.results:
        out += np.asarray(r["outp"], dtype=np.float32)
    return out
